# revision 27
# baseline (speedup 1.0000x reference)
"""GridRNN Trainium2 kernel.

Problem: 2-D grid RNN, B=4, S=T=128, H=256, D=3 depths.
  hx[d][b,i,j] = tanh(xin @ Wx_ih[d].T + bx_ih[d] + hx[d][b,i-1,(j-1)%T] @ Wx_hh[d].T + bx_hh[d])
  hy[d][b,i,j] = tanh(yin @ Wy_ih[d].T + by_ih[d] + hy[d][b,i,j-1]     @ Wy_hh[d].T + by_hh[d])
  (xin/yin = src/trg broadcast at d=0, previous depth's hx/hy for d>0)
  out = stack([hx[D-1], hy[D-1]], axis=-2)   # [B,S,T,2,H]

Key structure: the x-chain and y-chain never mix across depths -> 8 cores =
4 batches x 2 chains.  The x-chain's diagonal dependence hx[i-1,(j-1)%T] is
removed by shearing: u_i[c] = hx[i,(i+c)%T] turns it into a plain carry
u_{i-1}[c], identical in form to the y-chain.  One SPMD program runs on all
8 cores; only the input data differs per core.  The host unshears the x
outputs and transposes the y outputs.

Matmuls run in fp16 (1 cycle/row on the PE vs 4 for fp32); PSUM accumulates
fp32 and tanh reads the fp32 PSUM, so only operand rounding (~5e-4) enters
per step.  Depth-0's input term depends only on the step index; it is
precomputed on the HOST and injected as two pre0 rows of the rank-4 bias
matmul.

Performance model (HW-measured): the PE streams 1 column/cycle at 2.4GHz
only while the HAM activity monitor holds the clock gate at K=8/8; it cold-
starts at K=4/8 (half clock) and needs ~3.4us of gapless matmuls to lift.
An ACT instruction costs ~360-470ns nearly independent of size, so ACT
instruction COUNT is the other binding resource.  Hence:
 - a warm-up train of dummy matmuls runs while the const blob DMA streams,
 - two ACTs per tick: d0+d1 share one PSUM tile, their biases (b1 rows +
   per-step pre0 rows) enter via ONE rank-4 matmul (lhsT = 4 bias rows,
   rhs = block identity), and one fused [128,512] tanh writes both into a
   combined ring slot; d2 has its own [128,256] tanh (bias via rank-2),
 - d2 lags the wavefront by LAG2 ticks so all its inputs are old, keeping
   the PE stall-free,
 - this walrus build allows only ONE sync-wait per hardware instruction:
   every PSUM byte an ACT reads is written by the PE alone, and a post-pass
   moves any scheduler-induced second wait onto the preceding LDWEIGHTS.
"""

import numpy as np

import concourse.bass as bass
import concourse.tile as tile
from concourse import mybir
from concourse.bass_utils import run_bass_kernel_spmd

B, S, T, H, D = 4, 128, 128, 256, 3
P = 128          # partitions
K = H // P       # 2 k-tiles of H on partitions
F32 = mybir.dt.float32
F16 = mybir.dt.float16
TANH = mybir.ActivationFunctionType.Tanh

LAG2 = 4         # d2 wavefront lag (d0: t, d1: t-1, d2: t-LAG2)
NWARM = 100      # warm-up dummy matmuls (N=512): ~4.3us cold + ~19us warm,
                 # covers the HAM SHORT window (~3.4us) plus blob-DMA (~5MB)
NRING = 8        # u0/u1 ring depth (max lifetime = LAG2+1 ticks)

# blob column layout (fp32 words per partition)
# weights (fp16 via bitcast): 5 matrices (whh0, wih1, whh1, wih2, whh2)
NW = 5
WCW = NW * K * H // 2        # fp32 words used by weights
# bias4: per-step [4, 128] fp16 block on partitions 0-3 (lhsT base partition
# must be 0/32/64), one 128-col block per step -> S*128 fp16 = 8192 f32 cols
B4 = WCW
IND4 = B4 + 8192             # ind4 rhs [4p, 512] fp16 -> 256 f32 cols
B2 = IND4 + 256              # d2 bias rows [2p, 128] fp16 -> 64 f32 cols
CW = B2 + 64

_WSLOT = {(0, "hh"): 0, (1, "ih"): 1, (1, "hh"): 2, (2, "ih"): 3, (2, "hh"): 4}

OCHUNK = 32

_cache = {}


def _patched_drain_and_barrier(self, tick_clock, wait_clock):
    """Replacement for TileContext._drain_and_barrier.

    This walrus build lowers at most ONE sync-wait per instruction; the stock
    tail drain carries one wait per active proc.  Semantically the waits only
    need to complete before the final barrier's semaphore cleanup, so spread
    them over single-wait NOPs on the sync engine after the drain.
    """
    drain_inst = self.nc.sync.drain()
    wait_clock.add_sem_waits(
        drain_inst.ins, tile.ScopedClock({None: tick_clock.global_clock})
    )
    ins = drain_inst.ins
    si = ins.sync_info
    if si is not None and len(si.on_wait) > 1:
        waits = list(si.on_wait)
        ins.sync_info = mybir.SyncInfo(on_wait=[waits[0]],
                                       on_update=list(si.on_update))
        for w in waits[1:]:
            nop = self.nc.sync.nop(nofuse=True)
            nop.ins.sync_info = mybir.SyncInfo(on_wait=[w], on_update=[])

    self.nc.all_engine_barrier()
    assert self.sems is not None
    popped = self.nc._tile_sem_poison_stack.pop()
    assert popped is self._sem_poison
    self.nc.clear_and_free_semaphores(list(self.sems.allocated().values()))
    self.nc.all_engine_barrier()


tile.TileContext._drain_and_barrier = _patched_drain_and_barrier


def _split_multi_waits(nc):
    """Move excess sync-waits onto the preceding same-engine instruction.

    This walrus build allows one sync-wait per hardware instruction.  The
    Tile scheduler occasionally leaves an instruction with two (e.g. a
    reordered matmul waiting on both the ACT sem and the PE sem).  The
    engine queue is in-order, so a wait carried by the immediately preceding
    same-engine instruction gates the original instruction identically.
    """
    fn = nc.m.functions[0]
    insts = []

    def walk(block):
        for ins in block.instructions:
            insts.append(ins)
            for b in getattr(ins, "blocks", []) or []:
                walk(b)

    for bb in fn.blocks:
        walk(bb)

    prev_by_engine = {}
    upd_count = {}      # (engine, sem id) -> completed updates so far
    for ins in insts:
        eng = ins.engine
        si = ins.sync_info
        if si is not None and len(si.on_wait) > 1:
            waits = list(si.on_wait)
            # 1) drop same-engine self-waits already implied by serial order.
            #    ONLY safe on the Activation engine: ACT dispatches strictly
            #    serially (exec queue depth 0), so a self-sem threshold well
            #    behind its position is satisfied before it can issue.  The
            #    PE keeps up to 32 instructions in flight, so PE self-waits
            #    are load-bearing and must stay.
            if eng == mybir.EngineType.Activation:
                keep = []
                for w in waits:
                    n_done = upd_count.get((eng, w.id), 0)
                    if len(waits) - len(keep) > 1 and n_done >= w.wait_value + 4:
                        waits = [x for x in waits if x is not w]
                    else:
                        keep.append(w)
            # 2) move remaining excess onto the preceding same-engine
            #    zero-wait instruction (gates identically, in-order queue)
            while len(waits) > 1:
                carrier = prev_by_engine.get(eng)
                assert carrier is not None, "no wait carrier available"
                csi = carrier.sync_info
                assert csi is None or not csi.on_wait, (
                    f"carrier {carrier.name} already has waits")
                w = waits.pop(0)
                carrier.sync_info = mybir.SyncInfo(
                    on_wait=[w],
                    on_update=list(csi.on_update) if csi else [])
                prev_by_engine[eng] = None
            ins.sync_info = mybir.SyncInfo(on_wait=waits,
                                           on_update=list(si.on_update))
        si = ins.sync_info
        if si is not None:
            for u in si.on_update:
                key = (eng, u.id)
                upd_count[key] = upd_count.get(key, 0) + 1
        prev_by_engine[eng] = ins if (si is None or not si.on_wait) else None


def _build():
    nc = bass.Bass(trn_type="TRN2")

    blob = nc.dram_tensor("blob", [P, CW], F32, kind="ExternalInput")
    # DRAM layout mirrors the SBUF d2 plane ([p, s, k, v]); host reassembles
    # H = k*128+p.
    out = nc.dram_tensor("out", [P, S, K, T], F16, kind="ExternalOutput")
    out_c = out[:, :, :, :]

    with tile.TileContext(nc) as tc:
        with (
            tc.tile_pool(name="consts", bufs=1) as consts,
            tc.tile_pool(name="psA", bufs=2, space="PSUM") as psAp,
            tc.tile_pool(name="ps2", bufs=2, space="PSUM") as ps2p,
            tc.tile_pool(name="psi", bufs=1, space="PSUM") as psip,
        ):
            # garbage warm-up operands: zeroed SBUF, discarded PSUM.  Keeps
            # the PE gapless while the blob DMA streams so HAM lifts the
            # clock to K=8/8 before tick 0.
            warm = consts.tile([P, 512], F16)
            wps = psip.tile([P, 512], F32, tag="warm")
            nc.vector.memset(warm, 0.0)
            for i in range(NWARM):
                nc.tensor.matmul(wps[:, :], lhsT=warm[:, 0:P], rhs=warm[:, :],
                                 start=True, stop=True, skip_group_check=True)

            cb = consts.tile([P, CW], F32)
            nc.gpsimd.dma_start(out=cb, in_=blob[:, :])
            cb16 = cb[:, 0:WCW].bitcast(F16)
            bias4 = cb[:, B4:B4 + 8192].bitcast(F16)    # [128, 16384], rows 0-3
            ind4 = cb[:, IND4:IND4 + 256].bitcast(F16)  # [128, 512] rows 0-3
            b2 = cb[:, B2:B2 + 64].bitcast(F16)         # [128, 128] rows 0-1

            def w16(slot, k, m):
                c = (slot * K + k) * H + m * P
                return cb16[:, c:c + P]

            def wih(d, k, m):
                return w16(_WSLOT[(d, "ih")], k, m)

            def whh(d, k, m):
                return w16(_WSLOT[(d, "hh")], k, m)

            def bias4_lhsT(s):
                return bias4[0:4, s * P:(s + 1) * P]

            zeros = consts.tile([P, K, T], F16)
            nc.vector.memset(zeros, 0.0)
            # ScalarE absorber: folds the blob-DMA semaphore into ACT's clock
            scr = consts.tile([P, 4], F32)
            nc.scalar.copy(out=scr[:, 0:1], in_=cb[:, B2:B2 + 1])
            # PE absorber: folds the blob-DMA semaphore into PE's clock
            nc.tensor.matmul(wps[0:32, 0:32], lhsT=cb16[0:32, 0:32],
                             rhs=cb16[0:32, 0:32], start=True, stop=True,
                             skip_group_check=True)

            # ring slot t holds [u1(t-1) | u0(t)], each [K, T] fp16
            u01 = consts.tile([P, NRING, 2, K, T], F16)
            u2lin = consts.tile([P, S, K, T], F16)

            NT = S + LAG2
            for t in range(NT):
                d0_on = t < S
                d1_on = 1 <= t <= S

                if d0_on or d1_on:
                    psA = psAp.tile([P, 4, T], F32, tag="psA")
                    # rank-4 bias fill: [0:2]=d1 bias rows, [2:4]=pre0(t)
                    if d0_on:
                        nc.tensor.matmul(psA[:, :, :], lhsT=bias4_lhsT(t),
                                         rhs=ind4[0:4, :], start=True,
                                         stop=False, skip_group_check=True)
                    else:  # t == S: d1 bias only (rows 0-1 of any step block)
                        nc.tensor.matmul(psA[:, 0:2, :], lhsT=bias4[0:2, 0:P],
                                         rhs=ind4[0:2, 0:256], start=True,
                                         stop=False, skip_group_check=True)
                    if d0_on:
                        s = t
                        u_pr = zeros if s == 0 else u01[:, (t - 1) % NRING, 1, :, :]
                        for m in range(K):
                            for k in range(K):
                                nc.tensor.matmul(
                                    psA[:, 2 + m, :], lhsT=whh(0, k, m),
                                    rhs=u_pr[:, k, :], start=False,
                                    stop=(not d1_on and m == K - 1 and k == K - 1),
                                    skip_group_check=True)
                    if d1_on:
                        s = t - 1
                        sl = u01[:, (t - 1) % NRING, :, :, :]
                        u_in = sl[:, 1, :, :]                   # u0(t-1)
                        u_pr = zeros if s == 0 else sl[:, 0, :, :]  # u1(t-2)
                        for m in range(K):
                            for k in range(K):
                                nc.tensor.matmul(
                                    psA[:, m, :], lhsT=wih(1, k, m),
                                    rhs=u_in[:, k, :], start=False,
                                    stop=False, skip_group_check=True)
                            for k in range(K):
                                nc.tensor.matmul(
                                    psA[:, m, :], lhsT=whh(1, k, m),
                                    rhs=u_pr[:, k, :], start=False,
                                    stop=(m == K - 1 and k == K - 1),
                                    skip_group_check=True)
                    # fused tanh: writes [u1(t-1) | u0(t)] into ring slot t
                    if d0_on:
                        nc.scalar.activation(u01[:, t % NRING, :, :, :],
                                             psA[:, :, :], TANH, bias=0.0)
                    else:  # t == S: only the u1 half
                        nc.scalar.activation(u01[:, t % NRING, 0, :, :],
                                             psA[:, 0:2, :], TANH, bias=0.0)

                if LAG2 <= t:
                    s = t - LAG2
                    ps2 = ps2p.tile([P, K, T], F32, tag="ps2")
                    u_in = u01[:, (s + 1) % NRING, 0, :, :]     # u1(s)
                    u_pr = zeros if s == 0 else u2lin[:, s - 1, :, :]
                    # bias rank-2 FIRST: the single group opener for the
                    # whole zero region (a second start=True in an open
                    # region resets its has_written bits and drops values)
                    nc.tensor.matmul(ps2[:, :, :], lhsT=b2[0:2, :],
                                     rhs=ind4[0:2, 0:256], start=True,
                                     stop=False, skip_group_check=True)
                    for m in range(K):
                        for k in range(K):
                            nc.tensor.matmul(ps2[:, m, :], lhsT=wih(2, k, m),
                                             rhs=u_in[:, k, :], start=False,
                                             stop=False, skip_group_check=True)
                        for k in range(K):
                            nc.tensor.matmul(ps2[:, m, :], lhsT=whh(2, k, m),
                                             rhs=u_pr[:, k, :], start=False,
                                             stop=(m == K - 1 and k == K - 1),
                                             skip_group_check=True)
                    nc.scalar.activation(u2lin[:, s, :, :], ps2[:, :, :],
                                         TANH, bias=0.0)
                    if (s + 1) % OCHUNK == 0:
                        s0 = s + 1 - OCHUNK
                        nc.gpsimd.dma_start(
                            out=out_c[:, s0:s0 + OCHUNK, :, :],
                            in_=u2lin[:, s0:s0 + OCHUNK, :, :])

    _split_multi_waits(nc)
    return nc


def _blob(seed, wT_ih, wT_hh, bih, bhh):
    """Pack per-core constants into the [P, CW] blob.

    seed: [S, H] fp32; wT_ih/wT_hh: [D, H, H] (W[d].T); biases [D, H].
    """
    b = np.zeros((P, CW), np.float32)
    w = np.empty((NW, H, H), np.float32)
    w[0] = wT_hh[0]
    w[1], w[2] = wT_ih[1], wT_hh[1]
    w[3], w[4] = wT_ih[2], wT_hh[2]
    w16 = (w.reshape(NW, K, P, H).transpose(2, 0, 1, 3)
           .reshape(P, NW * K * H).astype(np.float16))
    b[:, 0:WCW] = w16.view(np.float32)
    bs = (bih + bhh).astype(np.float32)
    # pre0[s] = seed[s] @ W_ih[0] (wT_ih[0] is already W.T) + bias0
    p0 = (seed.astype(np.float32) @ wT_ih[0].astype(np.float32)
          + bs[0]).astype(np.float16)                    # [S, H]
    # bias4: for step s, lhsT rows 0-3 at col block s are
    # (b1m0, b1m1, pre0m0(s), pre0m1(s))
    b4 = np.zeros((P, S * P), np.float16)
    b1 = bs[1].astype(np.float16)
    for s in range(S):
        b4[0, s * P:(s + 1) * P] = b1[0:P]
        b4[1, s * P:(s + 1) * P] = b1[P:H]
        b4[2, s * P:(s + 1) * P] = p0[s, 0:P]
        b4[3, s * P:(s + 1) * P] = p0[s, P:H]
    b[:, B4:B4 + 8192] = b4.view(np.float32)
    # ind4: block identity [4, 512]
    i4 = np.zeros((P, 512), np.float16)
    for r in range(4):
        i4[r, r * 128:(r + 1) * 128] = 1.0
    b[:, IND4:IND4 + 256] = i4.view(np.float32)
    # d2 bias rows
    b2r = np.zeros((P, 128), np.float16)
    b2r[0, :] = bs[2, 0:P].astype(np.float16)
    b2r[1, :] = bs[2, P:H].astype(np.float16)
    b[:, B2:B2 + 64] = b2r.view(np.float32)
    return b


def kernel(src, trg, Wx_ih, Wx_hh, bx_ih, bx_hh, Wy_ih, Wy_hh, by_ih, by_hh):
    if "nc" not in _cache:
        _cache["nc"] = _build()
    nc = _cache["nc"]

    def tr(w):  # [D,H,H] -> W[d].T contiguous
        return np.ascontiguousarray(np.swapaxes(np.asarray(w, np.float32), 1, 2))

    src = np.asarray(src, np.float32)
    trg = np.asarray(trg, np.float32)
    wx_ihT, wx_hhT = tr(Wx_ih), tr(Wx_hh)
    wy_ihT, wy_hhT = tr(Wy_ih), tr(Wy_hh)
    bx_ih = np.asarray(bx_ih, np.float32)
    bx_hh = np.asarray(bx_hh, np.float32)
    by_ih = np.asarray(by_ih, np.float32)
    by_hh = np.asarray(by_hh, np.float32)

    in_maps = []
    for b in range(B):  # cores 0-3: x chains
        in_maps.append({"blob": _blob(src[b], wx_ihT, wx_hhT, bx_ih, bx_hh)})
    for b in range(B):  # cores 4-7: y chains
        in_maps.append({"blob": _blob(trg[b], wy_ihT, wy_hhT, by_ih, by_hh)})

    _cache["last_in_maps"] = in_maps
    globals()["_last_in_maps"] = in_maps
    res = run_bass_kernel_spmd(nc, in_maps, list(range(8)))

    out = np.empty((B, S, T, 2, H), np.float32)
    ii = np.arange(S)[:, None]
    jj = np.arange(T)[None, :]
    idx = (jj - ii) % T  # hx[i,j] = u_i[(j-i)%T]
    for b in range(B):
        # raw core output [p, s, k, v] -> [s, H=k*128+p, v]
        arr = (res.results[b]["out"].astype(np.float32)
               .transpose(1, 2, 0, 3).reshape(S, H, T))
        hx = np.take_along_axis(arr, idx[:, None, :], axis=2)  # [s, H, j]
        out[b, :, :, 0, :] = hx.transpose(0, 2, 1)
        arr = (res.results[B + b]["out"].astype(np.float32)
               .transpose(1, 2, 0, 3).reshape(S, H, T))
        out[b, :, :, 1, :] = arr.transpose(2, 0, 1)  # [j, H, i] -> [i, j, H]
    return out
